# revision 5
# baseline (speedup 1.0000x reference)
"""MAGNN metapath-specific GNN message passing on 8 Trainium2 NeuronCores.

Strategy (dst-sharding, no collectives):
  - Sort edges by destination node on host; core c owns dst nodes
    [c*12500, (c+1)*12500). Each destination's softmax segment lives
    entirely on one core, so no cross-core reduction is needed.
  - Host prepares (index manipulation only): gather indices, per-window
    one-hot scatter matrices, and the tiny (256x64 / 256x8) rotation +
    attention weight matrices A|B derived from r_vec/attn params.
  - Device, per window (<=128 nodes, exactly 384 edge slots = 3 chunks):
      indirect-gather edata [128e, 4*64] from features
      PE-transpose edata halves, matmul with A|B -> [hidden(64) | a(8)]
      leaky-relu (DVE) + exp (ACT) -> aexp
      rhs = hidden (x) aexp broadcast-mul (DVE) [128, 512]
      U += onehot^T @ rhs ; asum += onehot^T @ aexp  (PE, PSUM accum)
      evict: out = U * 1/max(asum,eps) fused on DVE, DMA to DRAM
  - Host unshards the padded window-major output back to [N, 8, 64].
"""

import sys

if "/opt/trn_rl_repo" not in sys.path:
    sys.path.insert(0, "/opt/trn_rl_repo")

import numpy as np

import concourse.bass as bass
import concourse.mybir as mybir
import concourse.tile as tile
from concourse import bacc
from concourse._compat import axon_active
from concourse.bass import IndirectOffsetOnAxis
from concourse.bass_utils import run_bass_kernel_spmd
from concourse.masks import make_identity

# Problem constants (hardcoded per spec)
ETYPES = [0, 2, 1]
D = 64
D2 = 32
H = 8
ALPHA = 0.01
R = 2
N = 100000
E = 250000
L = 4

NCORES = 8
NPC = N // NCORES  # nodes per core
P = 128
NSUB = 3  # chunks per window
EW = NSUB * P  # edge slots per window
QW = NSUB * L  # gather indices per partition per window


def _build_AB(r_vec, attn1_w, attn2):
    """A [256,64]: edata(rotated,meaned)->hidden.  B [256,8]: edata->attn logits."""
    rv = np.asarray(r_vec, np.float64)
    rv = rv / np.linalg.norm(rv, axis=2, keepdims=True)
    rv_conj = rv * np.array([1.0, -1.0])
    rv2 = np.stack((rv, rv_conj), axis=1).reshape(2 * R, D2, 2)
    fr = [None] * L
    fr[L - 1] = np.stack([np.ones(D2), np.zeros(D2)], -1)
    for i in range(L - 2, -1, -1):
        r = rv2[ETYPES[i]]
        re = fr[i + 1][:, 0] * r[:, 0] - fr[i + 1][:, 1] * r[:, 1]
        im = fr[i + 1][:, 0] * r[:, 1] + fr[i + 1][:, 1] * r[:, 0]
        fr[i] = np.stack([re, im], -1)
    frv = np.stack(fr, 0)  # [L, D2, 2]

    A = np.zeros((L * D, D))
    for l in range(L):
        c = frv[l, :, 0]
        s = frv[l, :, 1]
        for p in range(D2):
            A[l * D + 2 * p, 2 * p] = c[p] / L
            A[l * D + 2 * p, 2 * p + 1] = s[p] / L
            A[l * D + 2 * p + 1, 2 * p] = -s[p] / L
            A[l * D + 2 * p + 1, 2 * p + 1] = c[p] / L
    W1 = np.asarray(attn1_w, np.float64)  # [H, D]
    W2 = np.asarray(attn2, np.float64).reshape(H, D)
    B = A @ W2.T  # [256, H]
    B[(L - 1) * D : L * D, :] += W1.T  # center term (position L-1 raw feature)
    return A.astype(np.float32), B.astype(np.float32)


def _host_prep(features, emi, dst):
    """Sort/shard/window edges. Returns per-core arrays + window metadata."""
    emi = np.asarray(emi).astype(np.int32)
    dst = np.asarray(dst).astype(np.int64)
    deg = np.bincount(dst, minlength=N)
    assert deg.max() <= EW
    order = np.argsort(dst, kind="stable")
    dst_sorted = dst[order]

    core_wins = []  # per core: list of (node_base, span, estart, cnt)
    for c in range(NCORES):
        lo, hi = c * NPC, (c + 1) * NPC
        elo = int(np.searchsorted(dst_sorted, lo))
        wins = []
        node = lo
        eptr = elo
        while node < hi:
            base = node
            cnt = 0
            while node < hi and node - base < P:
                dn = int(deg[node])
                if cnt + dn > EW:
                    break
                cnt += dn
                node += 1
            wins.append((base, node - base, eptr, cnt))
            eptr += cnt
        core_wins.append(wins)

    NW = max(len(w) for w in core_wins)

    offs_all = []
    onehot_all = []
    for c in range(NCORES):
        wins = core_wins[c]
        nw = len(wins)
        base_w = np.zeros(NW, np.int64)
        cnt_w = np.zeros(NW, np.int64)
        est_w = np.zeros(NW, np.int64)
        for i, (b, sp, es, cn) in enumerate(wins):
            base_w[i], est_w[i], cnt_w[i] = b, es, cn

        slot = np.arange(EW)  # slot = s*128 + p
        valid = slot[None, :] < cnt_w[:, None]  # [NW, EW]
        eidx = np.minimum(est_w[:, None] + slot[None, :], E - 1)
        eids = order[eidx]  # [NW, EW] edge ids (garbage where invalid)

        gidx = emi[eids]  # [NW, EW, 4]
        gidx[~valid] = 0
        # offs layout: [128 p, NW, NSUB, L]  (slot = s*128+p -> [s, p])
        offs = gidx.reshape(NW, NSUB, P, L).transpose(2, 0, 1, 3)
        offs_all.append(np.ascontiguousarray(offs, np.int32).reshape(P, NW * QW))

        dl = dst[eids] - base_w[:, None]  # [NW, EW] local dst
        oh = np.zeros((NW, NSUB, P, P), np.float32)
        wI, sl = np.nonzero(valid)
        oh[wI, sl // P, sl % P, dl[wI, sl]] = 1.0
        # onehot dram layout: [NW*128 rows (w,p), NSUB*128]
        oh = oh.transpose(0, 2, 1, 3).reshape(NW * P, NSUB * P)
        onehot_all.append(np.ascontiguousarray(oh))

    return NW, core_wins, offs_all, onehot_all


_CACHE = {}


def _build_program(NW):
    if NW in _CACHE:
        return _CACHE[NW]
    fp32 = mybir.dt.float32
    nc = bacc.Bacc(
        "TRN2",
        target_bir_lowering=False,
        debug=not axon_active(),
        num_devices=NCORES,
    )
    feat_d = nc.dram_tensor("features", [N, D], fp32, kind="ExternalInput").ap()
    ab_d = nc.dram_tensor("ab", [P, 144], fp32, kind="ExternalInput").ap()
    offs_d = nc.dram_tensor("offs", [P, NW * QW], mybir.dt.int32, kind="ExternalInput").ap()
    oh_d = nc.dram_tensor("onehot", [NW * P, NSUB * P], fp32, kind="ExternalInput").ap()
    out_d = nc.dram_tensor("out", [NW * P, H * D], fp32, kind="ExternalOutput").ap()

    with tile.TileContext(nc) as tc:
        with (
            tc.tile_pool(name="const", bufs=1) as cpool,
            tc.tile_pool(name="oh", bufs=2) as ohpool,
            tc.tile_pool(name="ed", bufs=2) as edpool,
            tc.tile_pool(name="ts", bufs=3) as tspool,
            tc.tile_pool(name="sm", bufs=3) as smpool,
            tc.tile_pool(name="rhs", bufs=3) as rhspool,
            tc.tile_pool(name="ob", bufs=2) as obpool,
            tc.tile_pool(name="pt", bufs=4, space="PSUM") as ptpool,
            tc.tile_pool(name="pha", bufs=2, space="PSUM") as phapool,
            tc.tile_pool(name="pu", bufs=1, space="PSUM") as pupool,
            tc.tile_pool(name="pas", bufs=1, space="PSUM") as paspool,
        ):
            ident = cpool.tile([P, P], fp32, tag="ident")
            make_identity(nc, ident[:])
            ab_t = cpool.tile([P, 144], fp32, tag="ab")
            nc.sync.dma_start(out=ab_t[:], in_=ab_d[:, :])
            offs_t = cpool.tile([P, NW * QW], mybir.dt.int32, tag="offs")
            nc.sync.dma_start(out=offs_t[:], in_=offs_d[:, :])

            for w in range(NW):
                oh_t = ohpool.tile([P, NSUB * P], fp32, tag="oh")
                nc.sync.dma_start(out=oh_t[:], in_=oh_d[w * P : (w + 1) * P, :])
                ed_t = edpool.tile([P, QW * D], fp32, tag="ed")
                for q in range(QW):
                    nc.gpsimd.indirect_dma_start(
                        out=ed_t[:, q * D : (q + 1) * D],
                        out_offset=None,
                        in_=feat_d[:, :],
                        in_offset=IndirectOffsetOnAxis(
                            ap=offs_t[:, w * QW + q : w * QW + q + 1], axis=0
                        ),
                    )
                u_p = pupool.tile([P, H * D], fp32, tag="u")
                as_p = paspool.tile([P, H], fp32, tag="as")
                for s in range(NSUB):
                    ed_s = ed_t[:, s * (L * D) : (s + 1) * (L * D)]  # [128, 256]
                    t0_p = ptpool.tile([P, P], fp32, tag="tp")
                    t1_p = ptpool.tile([P, P], fp32, tag="tp")
                    nc.tensor.transpose(out=t0_p[:], in_=ed_s[:, 0:P], identity=ident[:])
                    nc.tensor.transpose(out=t1_p[:], in_=ed_s[:, P : 2 * P], identity=ident[:])
                    t0_s = tspool.tile([P, P], fp32, tag="tsb")
                    t1_s = tspool.tile([P, P], fp32, tag="tsb")
                    nc.scalar.activation(out=t0_s[:], in_=t0_p[:], func=mybir.ActivationFunctionType.Copy)
                    nc.scalar.activation(out=t1_s[:], in_=t1_p[:], func=mybir.ActivationFunctionType.Copy)
                    ha_p = phapool.tile([P, 72], fp32, tag="ha")
                    nc.tensor.matmul(out=ha_p[:], lhsT=t0_s[:], rhs=ab_t[:, 0:72], start=True, stop=False)
                    nc.tensor.matmul(out=ha_p[:], lhsT=t1_s[:], rhs=ab_t[:, 72:144], start=False, stop=True)
                    # leaky relu: max(a, 0.01*a)
                    t8 = smpool.tile([P, H], fp32, tag="t8")
                    nc.vector.tensor_scalar_mul(out=t8[:], in0=ha_p[:, D : D + H], scalar1=ALPHA)
                    alr = smpool.tile([P, H], fp32, tag="alr")
                    nc.vector.tensor_tensor(out=alr[:], in0=ha_p[:, D : D + H], in1=t8[:], op=mybir.AluOpType.max)
                    aexp = smpool.tile([P, H], fp32, tag="aexp")
                    nc.scalar.activation(out=aexp[:], in_=alr[:], func=mybir.ActivationFunctionType.Exp)
                    # rhs [128, 512] = hidden (h-bcast) * aexp (d-bcast)
                    rhs_t = rhspool.tile([P, H * D], fp32, tag="rhs")
                    nc.vector.tensor_tensor(
                        out=rhs_t[:].rearrange("p (h d) -> p h d", h=H),
                        in0=ha_p[:, 0:D].unsqueeze(1).to_broadcast([P, H, D]),
                        in1=aexp[:].unsqueeze(2).to_broadcast([P, H, D]),
                        op=mybir.AluOpType.mult,
                    )
                    oh_s = oh_t[:, s * P : (s + 1) * P]
                    nc.tensor.matmul(out=u_p[:], lhsT=oh_s, rhs=rhs_t[:], start=(s == 0), stop=(s == NSUB - 1), skip_group_check=True)
                    nc.tensor.matmul(out=as_p[:], lhsT=oh_s, rhs=aexp[:], start=(s == 0), stop=(s == NSUB - 1), skip_group_check=True)
                # epilogue: out = U / max(asum, eps)
                asafe = smpool.tile([P, H], fp32, tag="asafe")
                nc.vector.tensor_scalar_max(out=asafe[:], in0=as_p[:], scalar1=1e-20)
                rec = smpool.tile([P, H], fp32, tag="rec")
                nc.vector.reciprocal(out=rec[:], in_=asafe[:])
                o_sb = obpool.tile([P, H * D], fp32, tag="osb")
                nc.vector.tensor_tensor(
                    out=o_sb[:].rearrange("p (h d) -> p h d", h=H),
                    in0=u_p[:].rearrange("p (h d) -> p h d", h=H),
                    in1=rec[:].unsqueeze(2).to_broadcast([P, H, D]),
                    op=mybir.AluOpType.mult,
                )
                nc.sync.dma_start(out=out_d[w * P : (w + 1) * P, :], in_=o_sb[:])

    nc.compile()
    _CACHE[NW] = nc
    return nc


def kernel(features, r_vec, attn1_w, attn2, edge_metapath_indices, edge_dst):
    features = np.ascontiguousarray(np.asarray(features, np.float32))
    A, B = _build_AB(r_vec, attn1_w, attn2)
    ab = np.concatenate([A, B], axis=1)  # [256, 72]
    # ab dram layout [128, 144]: [:, 0:72] = rows 0:128, [:, 72:144] = rows 128:256
    ab_host = np.concatenate([ab[:P], ab[P:]], axis=1).astype(np.float32)
    ab_host = np.ascontiguousarray(ab_host)

    NW, core_wins, offs_all, onehot_all = _host_prep(
        features, edge_metapath_indices, edge_dst
    )
    nc = _build_program(NW)

    in_maps = []
    for c in range(NCORES):
        in_maps.append(
            {
                "features": features,
                "ab": ab_host,
                "offs": offs_all[c],
                "onehot": onehot_all[c],
            }
        )
    res = run_bass_kernel_spmd(nc, in_maps, core_ids=list(range(NCORES)))

    out = np.zeros((N, H, D), np.float32)
    for c in range(NCORES):
        co = res.results[c]["out"]  # [NW*128, 512]
        for w, (base, span, _es, _cn) in enumerate(core_wins[c]):
            if span > 0:
                out[base : base + span] = co[w * P : w * P + span].reshape(span, H, D)
    return out


if __name__ == "__main__":
    import reference

    inputs = {k: np.asarray(v) for k, v in reference.setup_inputs().items()}
    got = kernel(**inputs)
    print("kernel output", got.shape, got.dtype)
